# revision 24
# baseline (speedup 1.0000x reference)
"""Grouped linear (MoE grouped GEMM) on 8 TRN2 NeuronCores via Bass/Tile.

Reference: out = ragged_dot(x, weight.swapaxes(1,2), group_lens) with
x [32768, 1024] fp32, weight [16, 1024, 1024] fp32, tokens pre-sorted
into 16 contiguous groups.

Strategy -- token-parallel SPMD with host-side dispatch:
  * A deterministic hill-climb planner cuts each group's token run into
    chunks (one 2MB bf16 weight load each, up to 2 sub-slots of <=512
    tokens) and balances them across the 8 cores, minimizing
    max(PE stream, DMA bytes @ ~300GB/s).  All cores run ONE program
    shaped by the per-position maximum profile; per-core numpy inputs
    decide which expert/tokens each position processes.
  * Per sub-slot of width u: 8 out-blocks x 8 k-steps of [128x128] @
    [128xu] bf16 matmuls accumulated in fp32 PSUM, DVE PSUM->SBUF bf16
    casts, contiguous DMAs (weights on SP-HWDGE, x + outputs on
    ACT-HWDGE so queues never cross-block).
  * Head: 22 warm-up matmuls on junk keep the PE HAM clock-gate busy
    while piecewise first loads ([k0][k1-2][k3-4][k5-7]) land; slot 0
    runs k-outer/o-inner so the first matmuls need only the k0 pieces.
  * Tail: the last (narrowest) slot drains per o-block so only one
    64KB store trails the final matmul.

Measured on trn2 (8 cores, seed-0 data, single-shot NTFF span, max over
cores): ~137us, rel err 3.7e-3 (bf16 compute/output, fp32 accumulate).
Run-to-run +-3us; ~20% slower when the chip P0-downclocks to 2.0GHz.
"""

import numpy as np
import ml_dtypes

import concourse.bass as bass
import concourse.tile as tile
from concourse import bacc, mybir
from concourse.bass_utils import run_bass_kernel_spmd

G, NTOK, DIN, DOUT = 16, 32768, 1024, 1024
NCORES = 8
TT = 512           # max tokens per sub-slot
KT = DIN // 128    # 8 contraction sub-tiles
OB = DOUT // 128   # 8 output blocks
WALIGN = 16        # sub-slot width alignment (tokens)

_NC_CACHE: dict = {}


# ---------------------------------------------------------------- planner

def _split_even(total, maxpiece):
    np_ = -(-total // maxpiece)
    base = total // np_
    rem = total - base * np_
    return [base + (1 if i < rem else 0) for i in range(np_)]


def _split_cache():
    cache = {}
    def f(total):
        r = cache.get(total)
        if r is None:
            r = cache[total] = _split_even(total, TT) if total > 0 else []
        return r
    return f


def _profile_of(percore, split):
    key = lambda c: (-len(split(c[1])), -c[1])
    spc = [sorted(pc, key=key) for pc in percore]
    P = max(len(pc) for pc in spc)
    prof = []
    B = 0
    for p in range(P):
        subs = [split(pc[p][1]) if p < len(pc) else [] for pc in spc]
        m = max(len(s) for s in subs)
        widths = []
        for j in range(m):
            w = max((s[j] if j < len(s) else 0) for s in subs)
            w = -(-w // WALIGN) * WALIGN
            widths.append(w)
        prof.append(widths)
        B += sum(widths)
    return prof, B, P, spc


def _cost_of(percore, split):
    prof, B, P, _ = _profile_of(percore, split)
    pe = 64 * sum(max(u / 2.4 + 2.5, 110.0) for w in prof for u in w)
    dma = (P * 2 * 1024 * 1024 + B * 4096) / 0.30e3  # ~300 GB/s measured
    return max(pe, dma) + 13000.0


def _init_lpt(group_lens, cap):
    edges = np.concatenate([[0], np.cumsum(np.asarray(group_lens, np.int64))])
    chunks = []
    for g in range(G):
        for ln in _split_even(int(edges[g + 1] - edges[g]), cap) if edges[g + 1] > edges[g] else []:
            chunks.append((g, ln))
    chunks.sort(key=lambda c: -c[1])
    loads = [0] * NCORES
    percore = [[] for _ in range(NCORES)]
    for g, ln in chunks:
        i = min(range(NCORES), key=lambda i: (loads[i], len(percore[i])))
        loads[i] += ln
        percore[i].append((g, ln))
    return percore


def _hill_climb(percore, split, iters, seed):
    import random

    rng = random.Random(seed)
    cur = _cost_of(percore, split)
    for _ in range(iters):
        move = rng.random()
        pc = [list(x) for x in percore]
        if move < 0.35:
            a = rng.randrange(NCORES)
            b = rng.randrange(NCORES)
            if a == b or not pc[a]:
                continue
            pc[b].append(pc[a].pop(rng.randrange(len(pc[a]))))
        elif move < 0.6:
            g = rng.randrange(G)
            locs = [
                (c, i)
                for c in range(NCORES)
                for i, (gg, _) in enumerate(pc[c])
                if gg == g
            ]
            if len(locs) < 2:
                continue
            (c1, i1), (c2, i2) = rng.sample(locs, 2)
            l1, l2 = pc[c1][i1][1], pc[c2][i2][1]
            dl = rng.choice([16, 32, 64, 128, 256]) * rng.choice([1, -1])
            if l1 - dl <= 0 or l2 + dl <= 0:
                continue
            pc[c1][i1] = (g, l1 - dl)
            pc[c2][i2] = (g, l2 + dl)
        elif move < 0.75:
            c = rng.randrange(NCORES)
            if not pc[c]:
                continue
            i = rng.randrange(len(pc[c]))
            g, ln = pc[c][i]
            if ln < 2 * WALIGN:
                continue
            cut = rng.randrange(WALIGN, ln, WALIGN)
            pc[c][i] = (g, cut)
            pc[rng.randrange(NCORES)].append((g, ln - cut))
        elif move < 0.9:
            c = rng.randrange(NCORES)
            locs = {}
            for i, (g, _) in enumerate(pc[c]):
                locs.setdefault(g, []).append(i)
            gs = [g for g, v in locs.items() if len(v) >= 2]
            if not gs:
                continue
            g = rng.choice(gs)
            i1, i2 = locs[g][0], locs[g][1]
            ln = pc[c][i1][1] + pc[c][i2][1]
            for i in sorted((i1, i2), reverse=True):
                pc[c].pop(i)
            pc[c].append((g, ln))
        else:
            a = rng.randrange(NCORES)
            b = rng.randrange(NCORES)
            if a == b or not pc[a] or not pc[b]:
                continue
            i = rng.randrange(len(pc[a]))
            j = rng.randrange(len(pc[b]))
            pc[a][i], pc[b][j] = pc[b][j], pc[a][i]
        c2 = _cost_of(pc, split)
        if c2 <= cur:
            percore = pc
            cur = c2
    return percore, cur


_PLAN_CACHE: dict = {}


def _plan(group_lens):
    """Multi-subslot chunk plan: minimize max(PE stream, DMA) over chunk
    assignments (one weight load per chunk) via deterministic hill-climb."""
    gl_key = tuple(int(x) for x in np.asarray(group_lens).ravel())
    if gl_key in _PLAN_CACHE:
        return _PLAN_CACHE[gl_key]
    split = _split_cache()
    best = None
    for cap, seed, iters in ((768, 1, 30000), (1024, 1, 30000)):
        pc, cost = _hill_climb(_init_lpt(group_lens, cap), split, iters, seed)
        if best is None or cost < best[0]:
            best = (cost, pc)
    pc, cost = _hill_climb([list(x) for x in best[1]], split, 40000, 23)
    if cost < best[0]:
        best = (cost, pc)
    percore = best[1]
    prof, B, P, spc = _profile_of(percore, split)

    # Convert (g, len) chunks to contiguous token spans per expert.
    edges = np.concatenate([[0], np.cumsum(np.asarray(group_lens, np.int64))])
    heads = [int(edges[g]) for g in range(G)]
    assign = []
    for c in range(NCORES):
        row = []
        for p in range(P):
            if p >= len(spc[c]):
                row.append(None)
                continue
            g, ln = spc[c][p]
            tlist = []
            s = heads[g]
            for n in split(ln):
                tlist.append((int(s), int(n)))
                s += n
            heads[g] = s
            row.append((g, tlist))
        assign.append(row)
    profile = [list(w) for w in prof]
    _PLAN_CACHE[gl_key] = (profile, assign)
    return profile, assign


def _offsets(profile):
    xoff, ooff = [], []
    xl = ol = 0
    for widths in profile:
        xo, oo = [], []
        for u in widths:
            xo.append(xl)
            oo.append(ol)
            xl += KT * u
            ol += OB * u
        xoff.append(xo)
        ooff.append(oo)
    return xoff, ooff, xl, ol


# ------------------------------------------------------------- bass build

def _build(profile):
    key = tuple(tuple(w) for w in profile)
    if key in _NC_CACHE:
        return _NC_CACHE[key]
    dt_in = mybir.dt.bfloat16
    dt_out = mybir.dt.bfloat16
    f32 = mybir.dt.float32
    xoff, ooff, XL, OL = _offsets(profile)
    P = len(profile)

    nc = bacc.Bacc(None, target_bir_lowering=False)
    xt = nc.declare_dram_parameter("xt", [128, XL], dt_in, isOutput=False)
    wt = nc.declare_dram_parameter("wt", [128, P * KT * DOUT], dt_in, isOutput=False)
    ot = nc.declare_dram_parameter("ot", [128, OL], dt_out, isOutput=True)

    def cast(o, dst, src):
        # PSUM->SBUF casts: DVE only (GpSimd cannot access PSUM; the ACT
        # engine costs an ACT_TABLE_LOAD in the preamble + queue coupling)
        nc.vector.tensor_copy(dst, src)

    with tile.TileContext(nc) as tc:
        with (
            tc.tile_pool(name="wp", bufs=3) as wpool,
            tc.tile_pool(name="xp", bufs=3) as xpool,
            tc.tile_pool(name="op", bufs=3) as opool,
            tc.tile_pool(name="w0p", bufs=1) as w0pool,
            tc.tile_pool(name="x0p", bufs=1) as x0pool,
            tc.tile_pool(name="ps", bufs=8, space=bass.MemorySpace.PSUM) as pspool,
        ):
            # ---- PE warm-up: dummy matmuls while the first DMAs fly ----
            # HAM un-throttles (1.2 -> 2.4 GHz) after ~3.4us of sustained PE
            # activity; burn that window on junk so real matmuls start warm.
            wdum = w0pool.tile([128, 128], dt_in, tag="wdum", name="wdum")
            xdum = x0pool.tile([128, 256], dt_in, tag="xdum", name="xdum")
            nc.gpsimd.memset(wdum[:, :], 0)
            nc.gpsimd.memset(xdum[:, :], 0)
            psdum = pspool.tile([128, TT], f32, tag="ps", name="psdum")
            for i in range(22):
                nc.tensor.matmul(
                    psdum[:, :256], wdum[:, :], xdum[:, :], start=(i == 0), stop=(i == 21)
                )

            # ---- slot 0, sub-slot 0: fine-grained head ----
            # Piecewise first loads sized so transfers (not issue latency)
            # gate each k-step: weights on sync (SP-HWDGE) as k0 / k1-3 /
            # k4-7, x on scalar (ACT-HWDGE) as k0 / k1-7.  Small DMAs run
            # well below line rate, so pieces are as coarse as the compute
            # cadence allows.
            u0 = profile[0][0]
            w0p0 = w0pool.tile([128, DOUT], dt_in, tag="w0p0", name="w0p0")
            w0p1 = w0pool.tile([128, 2 * DOUT], dt_in, tag="w0p1", name="w0p1")
            w0p2 = w0pool.tile([128, 2 * DOUT], dt_in, tag="w0p2", name="w0p2")
            w0p3 = w0pool.tile([128, 3 * DOUT], dt_in, tag="w0p3", name="w0p3")
            x0p = [
                x0pool.tile([128, n * u0], dt_in, tag=f"x0p{i}", name=f"x0p{i}")
                for i, n in enumerate((1, 2, 2, 3))
            ]
            xb = xoff[0][0]
            nc.sync.dma_start(w0p0[:, :], wt[:, 0:DOUT])
            nc.scalar.dma_start(x0p[0][:, :], xt[:, xb : xb + u0])
            nc.sync.dma_start(w0p1[:, :], wt[:, DOUT : 3 * DOUT])
            nc.scalar.dma_start(x0p[1][:, :], xt[:, xb + u0 : xb + 3 * u0])
            nc.sync.dma_start(w0p2[:, :], wt[:, 3 * DOUT : 5 * DOUT])
            nc.scalar.dma_start(x0p[2][:, :], xt[:, xb + 3 * u0 : xb + 5 * u0])
            nc.sync.dma_start(w0p3[:, :], wt[:, 5 * DOUT : 8 * DOUT])
            nc.scalar.dma_start(x0p[3][:, :], xt[:, xb + 5 * u0 : xb + 8 * u0])

            def w0_slice(k, o):
                if k == 0:
                    return w0p0[:, o * 128 : (o + 1) * 128]
                if k < 3:
                    kk = k - 1
                    return w0p1[:, kk * DOUT + o * 128 : kk * DOUT + (o + 1) * 128]
                if k < 5:
                    kk = k - 3
                    return w0p2[:, kk * DOUT + o * 128 : kk * DOUT + (o + 1) * 128]
                kk = k - 5
                return w0p3[:, kk * DOUT + o * 128 : kk * DOUT + (o + 1) * 128]

            def x0_slice(k, u):
                if k == 0:
                    return x0p[0][:, :u]
                if k < 3:
                    return x0p[1][:, (k - 1) * u0 : (k - 1) * u0 + u]
                if k < 5:
                    return x0p[2][:, (k - 3) * u0 : (k - 3) * u0 + u]
                return x0p[3][:, (k - 5) * u0 : (k - 5) * u0 + u]

            ps0 = [
                pspool.tile([128, TT], f32, tag="ps", name=f"ps0_{o}")
                for o in range(OB)
            ]
            osb0 = opool.tile([128, OB * TT], dt_out, tag="osb", name="osb0")
            # k-outer, o-inner: the first 8 matmuls need only (w k0, x k0)
            for k in range(KT):
                for o in range(OB):
                    nc.tensor.matmul(
                        ps0[o][:, :u0],
                        w0_slice(k, o),
                        x0_slice(k, u0),
                        start=(k == 0),
                        stop=(k == KT - 1),
                    )
            for o in range(OB):
                cast(o, osb0[:, o * u0 : (o + 1) * u0], ps0[o][:, :u0])
            nc.scalar.dma_start(
                ot[:, ooff[0][0] : ooff[0][0] + OB * u0], osb0[:, : OB * u0]
            )

            # ---- remaining sub-slots / slots: steady-state pipeline ----
            for p, widths in enumerate(profile):
                jstart = 1 if p == 0 else 0
                if p > 0:
                    wsb = wpool.tile([128, KT * DOUT], dt_in, tag="wsb", name="wsb")
                    nc.sync.dma_start(
                        wsb[:, :], wt[:, p * KT * DOUT : (p + 1) * KT * DOUT]
                    )
                for j in range(jstart, len(widths)):
                    u = widths[j]
                    xsb = xpool.tile([128, KT * TT], dt_in, tag="xsb", name="xsb")
                    nc.scalar.dma_start(
                        xsb[:, : KT * u], xt[:, xoff[p][j] : xoff[p][j] + KT * u]
                    )
                    osb = opool.tile([128, OB * TT], dt_out, tag="osb", name="osb")
                    last = p == P - 1 and j == len(widths) - 1
                    for o in range(OB):
                        ps = pspool.tile([128, TT], f32, tag="ps", name="ps")
                        for k in range(KT):
                            if p == 0:
                                lhsT = w0_slice(k, o)
                            else:
                                lhsT = wsb[
                                    :, k * DOUT + o * 128 : k * DOUT + (o + 1) * 128
                                ]
                            nc.tensor.matmul(
                                ps[:, :u],
                                lhsT,
                                xsb[:, k * u : (k + 1) * u],
                                start=(k == 0),
                                stop=(k == KT - 1),
                            )
                        cast(o, osb[:, o * u : (o + 1) * u], ps[:, :u])
                        if last:
                            # per-o drain on the idle ACT queue: only the
                            # final o-block's 64KB trails the last matmul
                            nc.scalar.dma_start(
                                ot[:, ooff[p][j] + o * u : ooff[p][j] + (o + 1) * u],
                                osb[:, o * u : (o + 1) * u],
                            )
                    if not last:
                        nc.scalar.dma_start(
                            ot[:, ooff[p][j] : ooff[p][j] + OB * u], osb[:, : OB * u]
                        )

    nc.compile()
    _NC_CACHE[key] = nc
    return nc


# ----------------------------------------------------------- host scatter

def _prep_inputs(x, weight, profile, assign):
    xoff, ooff, XL, OL = _offsets(profile)
    P = len(profile)
    xbf = x.astype(ml_dtypes.bfloat16)
    # wpm[g][p, k*DOUT + o] = weight[g, o, k*128+p]
    wpm = np.ascontiguousarray(
        weight.reshape(G, DOUT, KT, 128).transpose(0, 3, 2, 1)
    ).astype(ml_dtypes.bfloat16).reshape(G, 128, KT * DOUT)
    in_maps = []
    for c in range(NCORES):
        xtc = np.zeros((128, XL), ml_dtypes.bfloat16)
        wtc = np.zeros((128, P * KT * DOUT), ml_dtypes.bfloat16)
        for p, widths in enumerate(profile):
            ch = assign[c][p]
            if ch is None:
                continue
            g, tlist = ch
            wtc[:, p * KT * DOUT : (p + 1) * KT * DOUT] = wpm[g]
            for j, (s, n) in enumerate(tlist):
                u = widths[j]
                b = np.zeros((u, DIN), ml_dtypes.bfloat16)
                b[:n] = xbf[s : s + n]
                xtc[:, xoff[p][j] : xoff[p][j] + KT * u] = (
                    b.reshape(u, KT, 128).transpose(2, 1, 0).reshape(128, KT * u)
                )
        in_maps.append({"xt": xtc, "wt": wtc})
    return in_maps


def _gather_out(results, profile, assign):
    xoff, ooff, XL, OL = _offsets(profile)
    out = np.empty((NTOK, DOUT), np.float32)
    for c in range(NCORES):
        otc = np.asarray(results[c]["ot"]).astype(np.float32)
        for p, widths in enumerate(profile):
            ch = assign[c][p]
            if ch is None:
                continue
            _, tlist = ch
            for j, (s, n) in enumerate(tlist):
                u = widths[j]
                blk = otc[:, ooff[p][j] : ooff[p][j] + OB * u].reshape(128, OB, u)
                out[s : s + n] = blk.transpose(2, 1, 0).reshape(u, DOUT)[:n]
    return out


def prepare(x, weight, group_lens):
    x = np.ascontiguousarray(np.asarray(x))
    weight = np.ascontiguousarray(np.asarray(weight))
    profile, assign = _plan(group_lens)
    nc = _build(profile)
    in_maps = _prep_inputs(x, weight, profile, assign)
    return nc, in_maps, lambda results: _gather_out(results, profile, assign)


def kernel(x, weight, group_lens):
    nc, in_maps, gather = prepare(x, weight, group_lens)
    res = run_bass_kernel_spmd(nc, in_maps, list(range(NCORES)))
    return gather(res.results)

